# revision 14
# baseline (speedup 1.0000x reference)
"""Bisect variant: baseline kernel + fp8 DoubleRow GEMMs only.

x scaled by 1/4, W1/W2 scaled by 4 into fp8e4 (product scale = 1, so no
activation descale needed). Everything else identical to the baseline.
"""

import numpy as np
import ml_dtypes

F8 = ml_dtypes.float8_e4m3
BF16 = ml_dtypes.bfloat16
N = 262144
D = 1024
H = 512
B = 512
NCORES = 8
ROWS = N // NCORES
P = 128
MAXB = 128
KC2 = D // 256              # 4 double-row chunks

_CACHE = {}
TRACE = False
LAST_RESULT = None


def _build_program(n_tiles):
    import concourse.bass as bass
    import concourse.bacc as bacc
    import concourse.mybir as mybir
    import concourse.tile as tile

    dt = mybir.dt
    AF = mybir.ActivationFunctionType
    ALU = mybir.AluOpType
    DR = mybir.MatmulPerfMode.DoubleRow

    nc = bacc.Bacc("TRN2", target_bir_lowering=False, debug=False,
                   num_devices=NCORES)

    rows = n_tiles * P
    xp = nc.dram_tensor("xp", (n_tiles // 2, P, 2 * D), dt.bfloat16,
                        kind="ExternalInput")
    # DR-packed transposed x: [t, p, 2c+i, m] = x[t*128+m, c*256+i*128+p]/4
    xpt = nc.dram_tensor("xpt", (n_tiles // 2, P, 2 * D), dt.float8e4,
                         kind="ExternalInput")
    oneh = nc.dram_tensor("oneh", (n_tiles // 2, P, 2 * MAXB), dt.bfloat16,
                          kind="ExternalInput")
    w12s = nc.dram_tensor("w12s", (P, 2 * KC2 * 2 * H), dt.float8e4,
                          kind="ExternalInput")
    w3r = nc.dram_tensor("w3r", (P, H), dt.float16, kind="ExternalInput")
    S = nc.dram_tensor("S", (MAXB, D), dt.float32, kind="ExternalOutput")
    E = nc.dram_tensor("E", (P, n_tiles), dt.float32, kind="ExternalOutput")

    with tile.TileContext(nc) as tc:
        with (
            tc.tile_pool(name="const", bufs=1) as constp,
            tc.tile_pool(name="xt", bufs=4) as xtp,
            tc.tile_pool(name="xn", bufs=4) as xnp_,
            tc.tile_pool(name="oh", bufs=4) as ohp,
            tc.tile_pool(name="work", bufs=4) as workp,
            tc.tile_pool(name="uvps", bufs=3, space=bass.MemorySpace.PSUM) as psp,
            tc.tile_pool(name="accps", bufs=1, space=bass.MemorySpace.PSUM) as psaccp,
        ):
            w12 = constp.tile([P, 2 * KC2, 2 * H], dt.float8e4)
            nc.gpsimd.dma_start(w12[:], w12s.ap())
            w3 = constp.tile([P, H], dt.float16)
            nc.gpsimd.dma_start(w3[:], w3r.ap())
            ebuf = constp.tile([P, n_tiles], dt.float32)
            pool_acc = psaccp.tile([MAXB, D], dt.float32)

            xn = oh = xt = None
            for t in range(n_tiles):
                if t % 2 == 0:
                    xt = xtp.tile([P, 2, 2 * KC2, P], dt.float8e4)
                    nc.gpsimd.dma_start(xt[:], xpt[t // 2])
                    xn = xnp_.tile([P, 2, D], dt.bfloat16)
                    nc.gpsimd.dma_start(xn[:], xp[t // 2])
                    oh = ohp.tile([P, 2, MAXB], dt.bfloat16)
                    nc.gpsimd.dma_start(oh[:], oneh[t // 2])

                uv = psp.tile([P, 2 * H], dt.float32)
                for c in range(KC2):
                    lhs = xt[:, t % 2, 2 * c:2 * c + 2, :]
                    nc.tensor.matmul(uv[:, 0:H], lhs,
                                     w12[:, 2 * c:2 * c + 2, 0:H],
                                     start=(c == 0), stop=(c == KC2 - 1),
                                     perf_mode=DR)
                    nc.tensor.matmul(uv[:, H:2 * H], lhs,
                                     w12[:, 2 * c:2 * c + 2, H:2 * H],
                                     start=(c == 0), stop=(c == KC2 - 1),
                                     perf_mode=DR)

                u16 = workp.tile([P, H], dt.float16)
                nc.scalar.activation(u16[:], uv[:, 0:H], AF.Tanh)
                ev = workp.tile([P, H], dt.float16)
                den = workp.tile([P, 1], dt.float32)
                nc.scalar.activation(ev[:], uv[:, H:2 * H], AF.Exp,
                                     accum_out=den[:])
                uw = workp.tile([P, H], dt.float16)
                nc.vector.tensor_tensor(uw[:], u16[:], w3[:], ALU.mult)
                prod = workp.tile([P, H], dt.float16)
                nc.vector.tensor_tensor(prod[:], uw[:], ev[:], ALU.mult)
                num = workp.tile([P, 1], dt.float32)
                nc.vector.reduce_sum(num[:], prod[:], mybir.AxisListType.X)
                rden = workp.tile([P, 1], dt.float32)
                nc.vector.reciprocal(rden[:], den[:])
                nc.scalar.activation(ebuf[:, t:t + 1], num[:], AF.Exp,
                                     scale=rden[:])
                lhsp = workp.tile([P, MAXB], dt.bfloat16)
                nc.vector.tensor_scalar_mul(lhsp[:], oh[:, t % 2, :],
                                            ebuf[:, t:t + 1])
                nc.tensor.matmul(pool_acc[:, 0:H], lhsp[:],
                                 xn[:, t % 2, 0:H],
                                 start=(t == 0), stop=(t == n_tiles - 1),
                                 skip_group_check=True)
                nc.tensor.matmul(pool_acc[:, H:D], lhsp[:],
                                 xn[:, t % 2, H:D],
                                 start=(t == 0), stop=(t == n_tiles - 1),
                                 skip_group_check=True)

            sout = constp.tile([MAXB, D], dt.float32)
            nc.scalar.copy(sout[:], pool_acc[:])
            nc.gpsimd.dma_start(S.ap(), sout[:])
            nc.gpsimd.dma_start(E.ap(), ebuf[:])

    nc.compile()
    return nc


def _get_program(n_tiles):
    if n_tiles not in _CACHE:
        _CACHE[n_tiles] = _build_program(n_tiles)
    return _CACHE[n_tiles]


def kernel(x, batch, W1, W2, W3):
    global LAST_RESULT
    from concourse import bass_utils

    x = np.asarray(x, dtype=np.float32)
    batch = np.asarray(batch)
    W1 = np.asarray(W1, dtype=np.float32)
    W2 = np.asarray(W2, dtype=np.float32)
    W3 = np.asarray(W3, dtype=np.float32)

    w12t = np.concatenate([W1.T, W2.T], axis=1)              # (D, 2H)
    w12dr = np.ascontiguousarray(
        (w12t * 4.0).reshape(KC2, 2, P, 2 * H).transpose(2, 0, 1, 3)
        .reshape(P, -1)).astype(F8)
    w3r = np.ascontiguousarray(
        np.broadcast_to(W3.reshape(1, H), (P, H))).astype(np.float16)

    x8 = (x * 0.25).astype(F8)
    x16 = x.astype(BF16)

    n_tiles = ROWS // P
    in_maps = []
    bases = []
    locals_ = []
    for c in range(NCORES):
        ids = batch[c * ROWS:(c + 1) * ROWS].astype(np.int64)
        base = int(ids[0])
        local = (ids - base).astype(np.int64)
        nb = int(local.max()) + 1
        assert nb <= MAXB, f"core {c}: {nb} local bags > {MAXB}"
        oneh = np.zeros((ROWS, MAXB), dtype=BF16)
        oneh[np.arange(ROWS), local] = BF16(1.0)
        xs8 = x8[c * ROWS:(c + 1) * ROWS]
        xt8 = np.ascontiguousarray(
            xs8.reshape(n_tiles // 2, 2, P, KC2, 2, P)
            .transpose(0, 5, 1, 3, 4, 2)
            .reshape(n_tiles // 2, P, 2 * D))
        xs16 = x16[c * ROWS:(c + 1) * ROWS]
        xp2 = np.ascontiguousarray(
            xs16.reshape(n_tiles // 2, 2, P, D).transpose(0, 2, 1, 3)
            .reshape(n_tiles // 2, P, 2 * D))
        oh2 = np.ascontiguousarray(
            oneh.reshape(n_tiles // 2, 2, P, MAXB).transpose(0, 2, 1, 3)
            .reshape(n_tiles // 2, P, 2 * MAXB))
        in_maps.append({
            "xp": xp2,
            "xpt": xt8,
            "oneh": oh2,
            "w12s": w12dr,
            "w3r": w3r,
        })
        bases.append(base)
        locals_.append(local)

    nc = _get_program(n_tiles)
    res = bass_utils.run_bass_kernel_spmd(
        nc, in_maps, core_ids=list(range(NCORES)), trace=TRACE)
    LAST_RESULT = res

    Z = np.zeros((B, D), dtype=np.float64)
    DEN = np.zeros((B,), dtype=np.float64)
    for c in range(NCORES):
        Sc = np.asarray(res.results[c]["S"], dtype=np.float64)
        Ec = np.asarray(res.results[c]["E"], dtype=np.float64)
        e_flat = Ec.T.reshape(-1)
        local = locals_[c]
        nb = int(local.max()) + 1
        den = np.bincount(local, weights=e_flat, minlength=nb)[:nb]
        Z[bases[c]:bases[c] + nb] += Sc[:nb]
        DEN[bases[c]:bases[c] + nb] += den
    out = np.zeros((B, D), dtype=np.float32)
    nzero = DEN > 0
    out[nzero] = (Z[nzero] / DEN[nzero, None]).astype(np.float32)
    return out


# revision 20
# speedup vs baseline: 1.1847x; 1.1847x over previous
"""GatedAttentionPooling Trainium2 kernel (594 us on 8 cores).

z[b] = sum_{i in bag b} softmax_bag(alpha)_i * x_i
alpha_i = (tanh(x W1^T) * softmax_h(x W2^T)) @ W3^T

Data-parallel over 8 cores (contiguous row split; sorted batch ids).
Per core, per 128-row tile:
  - the two big GEMMs run in fp8e4 DoubleRow mode (2x PE throughput);
    x scaled by 1/4 and W1/W2 by 4 into fp8 so the product scale is 1
    and no activation descale is needed
  - ACT tanh / exp (exp with fused row-sum accumulator), DVE ops for
    the w3-weighted numerator; e = exp(num/den) written directly into
    the E staging buffer column, which also feeds the onehot scaling
  - pooling matmul (bf16): (onehot * e)^T @ x accumulated in PSUM over
    all tiles; x/onehot pooling operands are DMA'd in 2-tile pairs
Host merges per-core partial sums and exp-sums linearly (exact).
"""

import numpy as np
import ml_dtypes

F8 = ml_dtypes.float8_e4m3
BF16 = ml_dtypes.bfloat16
N = 262144
D = 1024
H = 512
B = 512
NCORES = 8
ROWS = N // NCORES
P = 128
MAXB = 128
KC2 = D // 256              # 4 double-row chunks

_CACHE = {}
TRACE = False
LAST_RESULT = None


def _build_program(n_tiles):
    import concourse.bass as bass
    import concourse.bacc as bacc
    import concourse.mybir as mybir
    import concourse.tile as tile

    dt = mybir.dt
    AF = mybir.ActivationFunctionType
    ALU = mybir.AluOpType
    DR = mybir.MatmulPerfMode.DoubleRow

    nc = bacc.Bacc("TRN2", target_bir_lowering=False, debug=False,
                   num_devices=NCORES)

    rows = n_tiles * P
    xp = nc.dram_tensor("xp", (n_tiles // 2, P, 2 * D), dt.bfloat16,
                        kind="ExternalInput")
    # DR-packed transposed x: [t, p, 2c+i, m] = x[t*128+m, c*256+i*128+p]/4
    xpt = nc.dram_tensor("xpt", (n_tiles, P, D), dt.float8e4,
                         kind="ExternalInput")
    oneh = nc.dram_tensor("oneh", (n_tiles // 2, P, 2 * MAXB), dt.bfloat16,
                          kind="ExternalInput")
    w12s = nc.dram_tensor("w12s", (P, 2 * KC2 * 2 * H), dt.float8e4,
                          kind="ExternalInput")
    w3r = nc.dram_tensor("w3r", (P, H), dt.float16, kind="ExternalInput")
    S = nc.dram_tensor("S", (MAXB, D), dt.float32, kind="ExternalOutput")
    E = nc.dram_tensor("E", (P, n_tiles), dt.float32, kind="ExternalOutput")

    with tile.TileContext(nc) as tc:
        with (
            tc.tile_pool(name="const", bufs=1) as constp,
            tc.tile_pool(name="xt", bufs=4) as xtp,
            tc.tile_pool(name="xn", bufs=4) as xnp_,
            tc.tile_pool(name="oh", bufs=4) as ohp,
            tc.tile_pool(name="work", bufs=6) as workp,
            tc.tile_pool(name="uvps", bufs=3, space=bass.MemorySpace.PSUM) as psp,
            tc.tile_pool(name="accps", bufs=1, space=bass.MemorySpace.PSUM) as psaccp,
        ):
            w12 = constp.tile([P, 2 * KC2, 2 * H], dt.float8e4)
            nc.gpsimd.dma_start(w12[:], w12s.ap())
            w3 = constp.tile([P, H], dt.float16)
            nc.gpsimd.dma_start(w3[:], w3r.ap())
            ebuf = constp.tile([P, n_tiles], dt.float32)
            pool_acc = psaccp.tile([MAXB, D], dt.float32)

            # pool matmuls are emitted DELAY tiles late so the in-order
            # tensor queue never stalls waiting on the ACT/DVE chain that
            # produces lhsp
            DELAY = 2
            pend = {}

            def emit_pool(idx):
                l2, x2, j2 = pend.pop(idx)
                nc.tensor.matmul(pool_acc[:, 0:H], l2[:], x2[:, j2, 0:H],
                                 start=(idx == 0), stop=(idx == n_tiles - 1),
                                 skip_group_check=True)
                nc.tensor.matmul(pool_acc[:, H:D], l2[:], x2[:, j2, H:D],
                                 start=(idx == 0), stop=(idx == n_tiles - 1),
                                 skip_group_check=True)

            xn = oh = None
            for t in range(n_tiles):
                xt = xtp.tile([P, 2 * KC2, P], dt.float8e4)
                nc.gpsimd.dma_start(xt[:], xpt[t])
                if t % 2 == 0:
                    xn = xnp_.tile([P, 2, D], dt.bfloat16)
                    nc.gpsimd.dma_start(xn[:], xp[t // 2])
                    oh = ohp.tile([P, 2, MAXB], dt.bfloat16)
                    nc.gpsimd.dma_start(oh[:], oneh[t // 2])

                uv = psp.tile([P, 2 * H], dt.float32)
                for c in range(KC2):
                    lhs = xt[:, 2 * c:2 * c + 2, :]
                    nc.tensor.matmul(uv[:, 0:H], lhs,
                                     w12[:, 2 * c:2 * c + 2, 0:H],
                                     start=(c == 0), stop=(c == KC2 - 1),
                                     perf_mode=DR)
                    nc.tensor.matmul(uv[:, H:2 * H], lhs,
                                     w12[:, 2 * c:2 * c + 2, H:2 * H],
                                     start=(c == 0), stop=(c == KC2 - 1),
                                     perf_mode=DR)
                if t >= DELAY:
                    emit_pool(t - DELAY)

                u16 = workp.tile([P, H], dt.float16)
                nc.scalar.activation(u16[:], uv[:, 0:H], AF.Tanh)
                ev = workp.tile([P, H], dt.float16)
                den = workp.tile([P, 1], dt.float32)
                nc.scalar.activation(ev[:], uv[:, H:2 * H], AF.Exp,
                                     accum_out=den[:])
                uw = workp.tile([P, H], dt.float16)
                nc.vector.tensor_tensor(uw[:], u16[:], w3[:], ALU.mult)
                prod = workp.tile([P, H], dt.float16)
                nc.vector.tensor_tensor(prod[:], uw[:], ev[:], ALU.mult)
                num = workp.tile([P, 1], dt.float32)
                nc.vector.reduce_sum(num[:], prod[:], mybir.AxisListType.X)
                rden = workp.tile([P, 1], dt.float32)
                nc.vector.reciprocal(rden[:], den[:])
                nc.scalar.activation(ebuf[:, t:t + 1], num[:], AF.Exp,
                                     scale=rden[:])
                lhsp = workp.tile([P, MAXB], dt.bfloat16)
                nc.vector.tensor_scalar_mul(lhsp[:], oh[:, t % 2, :],
                                            ebuf[:, t:t + 1])
                pend[t] = (lhsp, xn, t % 2)

            for idx in range(n_tiles - DELAY, n_tiles):
                emit_pool(idx)

            sout = constp.tile([MAXB, D], dt.float32)
            nc.scalar.copy(sout[:], pool_acc[:])
            nc.gpsimd.dma_start(S.ap(), sout[:])
            nc.gpsimd.dma_start(E.ap(), ebuf[:])

    nc.compile()
    return nc


def _get_program(n_tiles):
    if n_tiles not in _CACHE:
        _CACHE[n_tiles] = _build_program(n_tiles)
    return _CACHE[n_tiles]


def kernel(x, batch, W1, W2, W3):
    global LAST_RESULT
    from concourse import bass_utils

    x = np.asarray(x, dtype=np.float32)
    batch = np.asarray(batch)
    W1 = np.asarray(W1, dtype=np.float32)
    W2 = np.asarray(W2, dtype=np.float32)
    W3 = np.asarray(W3, dtype=np.float32)

    w12t = np.concatenate([W1.T, W2.T], axis=1)              # (D, 2H)
    w12dr = np.ascontiguousarray(
        (w12t * 4.0).reshape(KC2, 2, P, 2 * H).transpose(2, 0, 1, 3)
        .reshape(P, -1)).astype(F8)
    w3r = np.ascontiguousarray(
        np.broadcast_to(W3.reshape(1, H), (P, H))).astype(np.float16)

    x8 = (x * 0.25).astype(F8)
    x16 = x.astype(BF16)

    n_tiles = ROWS // P
    in_maps = []
    bases = []
    locals_ = []
    for c in range(NCORES):
        ids = batch[c * ROWS:(c + 1) * ROWS].astype(np.int64)
        base = int(ids[0])
        local = (ids - base).astype(np.int64)
        nb = int(local.max()) + 1
        assert nb <= MAXB, f"core {c}: {nb} local bags > {MAXB}"
        oneh = np.zeros((ROWS, MAXB), dtype=BF16)
        oneh[np.arange(ROWS), local] = BF16(1.0)
        xs8 = x8[c * ROWS:(c + 1) * ROWS]
        xt8 = np.ascontiguousarray(
            xs8.reshape(n_tiles, P, KC2, 2, P).transpose(0, 4, 2, 3, 1)
            .reshape(n_tiles, P, D))
        xs16 = x16[c * ROWS:(c + 1) * ROWS]
        xp2 = np.ascontiguousarray(
            xs16.reshape(n_tiles // 2, 2, P, D).transpose(0, 2, 1, 3)
            .reshape(n_tiles // 2, P, 2 * D))
        oh2 = np.ascontiguousarray(
            oneh.reshape(n_tiles // 2, 2, P, MAXB).transpose(0, 2, 1, 3)
            .reshape(n_tiles // 2, P, 2 * MAXB))
        in_maps.append({
            "xp": xp2,
            "xpt": xt8,
            "oneh": oh2,
            "w12s": w12dr,
            "w3r": w3r,
        })
        bases.append(base)
        locals_.append(local)

    nc = _get_program(n_tiles)
    res = bass_utils.run_bass_kernel_spmd(
        nc, in_maps, core_ids=list(range(NCORES)), trace=TRACE)
    LAST_RESULT = res

    Z = np.zeros((B, D), dtype=np.float64)
    DEN = np.zeros((B,), dtype=np.float64)
    for c in range(NCORES):
        Sc = np.asarray(res.results[c]["S"], dtype=np.float64)
        Ec = np.asarray(res.results[c]["E"], dtype=np.float64)
        e_flat = Ec.T.reshape(-1)
        local = locals_[c]
        nb = int(local.max()) + 1
        den = np.bincount(local, weights=e_flat, minlength=nb)[:nb]
        Z[bases[c]:bases[c] + nb] += Sc[:nb]
        DEN[bases[c]:bases[c] + nb] += den
    out = np.zeros((B, D), dtype=np.float32)
    nzero = DEN > 0
    out[nzero] = (Z[nzero] / DEN[nzero, None]).astype(np.float32)
    return out


# revision 23
# speedup vs baseline: 1.2024x; 1.0149x over previous
"""GatedAttentionPooling Trainium2 kernel (594102 ns on 8 cores).

z[b] = sum_{i in bag b} softmax_bag(alpha)_i * x_i
alpha_i = (tanh(x W1^T) * softmax_h(x W2^T)) @ W3^T

Data-parallel over 8 cores (contiguous row split; sorted batch ids).
Per core, per 128-row tile:
  - the two big GEMMs run in fp8e4 DoubleRow mode (2x PE throughput);
    x scaled by 1/4 and W1/W2 by 4 into fp8 so the product scale is 1
    and no activation descale is needed
  - ACT tanh / exp (exp with fused row-sum accumulator), DVE ops for
    the w3-weighted numerator; e = exp(num/den) written directly into
    the E staging buffer column, which also feeds the onehot scaling
  - pooling matmul (bf16): (onehot * e)^T @ x accumulated in PSUM over
    all tiles; x/onehot pooling operands are DMA'd in 2-tile pairs
Host merges per-core partial sums and exp-sums linearly (exact).
"""

import numpy as np
import ml_dtypes

F8 = ml_dtypes.float8_e4m3
BF16 = ml_dtypes.bfloat16
N = 262144
D = 1024
H = 512
B = 512
NCORES = 8
ROWS = N // NCORES
P = 128
MAXB = 128
KC2 = D // 256              # 4 double-row chunks

_CACHE = {}
TRACE = False
LAST_RESULT = None


def _build_program(n_tiles):
    import concourse.bass as bass
    import concourse.bacc as bacc
    import concourse.mybir as mybir
    import concourse.tile as tile

    dt = mybir.dt
    AF = mybir.ActivationFunctionType
    ALU = mybir.AluOpType
    DR = mybir.MatmulPerfMode.DoubleRow

    nc = bacc.Bacc("TRN2", target_bir_lowering=False, debug=False,
                   num_devices=NCORES)

    rows = n_tiles * P
    xp = nc.dram_tensor("xp", (n_tiles // 2, P, 2 * D), dt.bfloat16,
                        kind="ExternalInput")
    # DR-packed transposed x: [t, p, 2c+i, m] = x[t*128+m, c*256+i*128+p]/4
    xpt = nc.dram_tensor("xpt", (n_tiles, P, D), dt.float8e4,
                         kind="ExternalInput")
    oneh = nc.dram_tensor("oneh", (n_tiles // 2, P, 2 * MAXB), dt.bfloat16,
                          kind="ExternalInput")
    w12s = nc.dram_tensor("w12s", (P, 2 * KC2 * 2 * H), dt.float8e4,
                          kind="ExternalInput")
    w3r = nc.dram_tensor("w3r", (P, H), dt.float16, kind="ExternalInput")
    S = nc.dram_tensor("S", (MAXB, D), dt.float32, kind="ExternalOutput")
    E = nc.dram_tensor("E", (P, n_tiles), dt.float32, kind="ExternalOutput")

    with tile.TileContext(nc) as tc:
        with (
            tc.tile_pool(name="const", bufs=1) as constp,
            tc.tile_pool(name="xt", bufs=8) as xtp,
            tc.tile_pool(name="xn", bufs=6) as xnp_,
            tc.tile_pool(name="oh", bufs=6) as ohp,
            tc.tile_pool(name="work", bufs=4) as workp,
            tc.tile_pool(name="uvps", bufs=3, space=bass.MemorySpace.PSUM) as psp,
            tc.tile_pool(name="accps", bufs=1, space=bass.MemorySpace.PSUM) as psaccp,
        ):
            w12 = constp.tile([P, 2 * KC2, 2 * H], dt.float8e4)
            nc.gpsimd.dma_start(w12[:], w12s.ap())
            w3 = constp.tile([P, H], dt.float16)
            nc.gpsimd.dma_start(w3[:], w3r.ap())
            ebuf = constp.tile([P, n_tiles], dt.float32)
            pool_acc = psaccp.tile([MAXB, D], dt.float32)

            xn = oh = None
            for t in range(n_tiles):
                xt = xtp.tile([P, 2 * KC2, P], dt.float8e4)
                nc.sync.dma_start(xt[:], xpt[t])
                if t % 2 == 0:
                    xn = xnp_.tile([P, 2, D], dt.bfloat16)
                    nc.gpsimd.dma_start(xn[:], xp[t // 2])
                    oh = ohp.tile([P, 2, MAXB], dt.bfloat16)
                    nc.gpsimd.dma_start(oh[:], oneh[t // 2])

                uv = psp.tile([P, 2 * H], dt.float32)
                for c in range(KC2):
                    lhs = xt[:, 2 * c:2 * c + 2, :]
                    nc.tensor.matmul(uv[:, 0:H], lhs,
                                     w12[:, 2 * c:2 * c + 2, 0:H],
                                     start=(c == 0), stop=(c == KC2 - 1),
                                     perf_mode=DR)
                    nc.tensor.matmul(uv[:, H:2 * H], lhs,
                                     w12[:, 2 * c:2 * c + 2, H:2 * H],
                                     start=(c == 0), stop=(c == KC2 - 1),
                                     perf_mode=DR)

                u16 = workp.tile([P, H], dt.float16)
                nc.scalar.activation(u16[:], uv[:, 0:H], AF.Tanh)
                ev = workp.tile([P, H], dt.float16)
                den = workp.tile([P, 1], dt.float32)
                nc.scalar.activation(ev[:], uv[:, H:2 * H], AF.Exp,
                                     accum_out=den[:])
                uw = workp.tile([P, H], dt.float16)
                nc.vector.tensor_tensor(uw[:], u16[:], w3[:], ALU.mult)
                prod = workp.tile([P, H], dt.float16)
                nc.vector.tensor_tensor(prod[:], uw[:], ev[:], ALU.mult)
                num = workp.tile([P, 1], dt.float32)
                nc.vector.reduce_sum(num[:], prod[:], mybir.AxisListType.X)
                rden = workp.tile([P, 1], dt.float32)
                nc.vector.reciprocal(rden[:], den[:])
                nc.scalar.activation(ebuf[:, t:t + 1], num[:], AF.Exp,
                                     scale=rden[:])
                lhsp = workp.tile([P, MAXB], dt.bfloat16)
                nc.vector.tensor_scalar_mul(lhsp[:], oh[:, t % 2, :],
                                            ebuf[:, t:t + 1])
                nc.tensor.matmul(pool_acc[:, 0:H], lhsp[:],
                                 xn[:, t % 2, 0:H],
                                 start=(t == 0), stop=(t == n_tiles - 1),
                                 skip_group_check=True)
                nc.tensor.matmul(pool_acc[:, H:D], lhsp[:],
                                 xn[:, t % 2, H:D],
                                 start=(t == 0), stop=(t == n_tiles - 1),
                                 skip_group_check=True)

            sout = constp.tile([MAXB, D], dt.float32)
            nc.scalar.copy(sout[:], pool_acc[:])
            nc.gpsimd.dma_start(S.ap(), sout[:])
            nc.gpsimd.dma_start(E.ap(), ebuf[:])

    nc.compile()
    return nc


def _get_program(n_tiles):
    if n_tiles not in _CACHE:
        _CACHE[n_tiles] = _build_program(n_tiles)
    return _CACHE[n_tiles]


def kernel(x, batch, W1, W2, W3):
    global LAST_RESULT
    from concourse import bass_utils

    x = np.asarray(x, dtype=np.float32)
    batch = np.asarray(batch)
    W1 = np.asarray(W1, dtype=np.float32)
    W2 = np.asarray(W2, dtype=np.float32)
    W3 = np.asarray(W3, dtype=np.float32)

    w12t = np.concatenate([W1.T, W2.T], axis=1)              # (D, 2H)
    w12dr = np.ascontiguousarray(
        (w12t * 4.0).reshape(KC2, 2, P, 2 * H).transpose(2, 0, 1, 3)
        .reshape(P, -1)).astype(F8)
    w3r = np.ascontiguousarray(
        np.broadcast_to(W3.reshape(1, H), (P, H))).astype(np.float16)

    x8 = (x * 0.25).astype(F8)
    x16 = x.astype(BF16)

    n_tiles = ROWS // P
    in_maps = []
    bases = []
    locals_ = []
    for c in range(NCORES):
        ids = batch[c * ROWS:(c + 1) * ROWS].astype(np.int64)
        base = int(ids[0])
        local = (ids - base).astype(np.int64)
        nb = int(local.max()) + 1
        assert nb <= MAXB, f"core {c}: {nb} local bags > {MAXB}"
        oneh = np.zeros((ROWS, MAXB), dtype=BF16)
        oneh[np.arange(ROWS), local] = BF16(1.0)
        xs8 = x8[c * ROWS:(c + 1) * ROWS]
        xt8 = np.ascontiguousarray(
            xs8.reshape(n_tiles, P, KC2, 2, P).transpose(0, 4, 2, 3, 1)
            .reshape(n_tiles, P, D))
        xs16 = x16[c * ROWS:(c + 1) * ROWS]
        xp2 = np.ascontiguousarray(
            xs16.reshape(n_tiles // 2, 2, P, D).transpose(0, 2, 1, 3)
            .reshape(n_tiles // 2, P, 2 * D))
        oh2 = np.ascontiguousarray(
            oneh.reshape(n_tiles // 2, 2, P, MAXB).transpose(0, 2, 1, 3)
            .reshape(n_tiles // 2, P, 2 * MAXB))
        in_maps.append({
            "xp": xp2,
            "xpt": xt8,
            "oneh": oh2,
            "w12s": w12dr,
            "w3r": w3r,
        })
        bases.append(base)
        locals_.append(local)

    nc = _get_program(n_tiles)
    res = bass_utils.run_bass_kernel_spmd(
        nc, in_maps, core_ids=list(range(NCORES)), trace=TRACE)
    LAST_RESULT = res

    Z = np.zeros((B, D), dtype=np.float64)
    DEN = np.zeros((B,), dtype=np.float64)
    for c in range(NCORES):
        Sc = np.asarray(res.results[c]["S"], dtype=np.float64)
        Ec = np.asarray(res.results[c]["E"], dtype=np.float64)
        e_flat = Ec.T.reshape(-1)
        local = locals_[c]
        nb = int(local.max()) + 1
        den = np.bincount(local, weights=e_flat, minlength=nb)[:nb]
        Z[bases[c]:bases[c] + nb] += Sc[:nb]
        DEN[bases[c]:bases[c] + nb] += den
    out = np.zeros((B, D), dtype=np.float32)
    nzero = DEN > 0
    out[nzero] = (Z[nzero] / DEN[nzero, None]).astype(np.float32)
    return out


# revision 26
# speedup vs baseline: 1.2047x; 1.0020x over previous
"""GatedAttentionPooling Trainium2 kernel (594102 ns on 8 cores).

z[b] = sum_{i in bag b} softmax_bag(alpha)_i * x_i
alpha_i = (tanh(x W1^T) * softmax_h(x W2^T)) @ W3^T

Data-parallel over 8 cores (contiguous row split; sorted batch ids).
Per core, per 128-row tile:
  - the two big GEMMs run in fp8e4 DoubleRow mode (2x PE throughput);
    x scaled by 1/4 and W1/W2 by 4 into fp8 so the product scale is 1
    and no activation descale is needed
  - ACT tanh / exp (exp with fused row-sum accumulator), DVE ops for
    the w3-weighted numerator; e = exp(num/den) written directly into
    the E staging buffer column, which also feeds the onehot scaling
  - pooling matmul (bf16): (onehot * e)^T @ x accumulated in PSUM over
    all tiles; x/onehot pooling operands are DMA'd in 2-tile pairs
Host merges per-core partial sums and exp-sums linearly (exact).
"""

import numpy as np
import ml_dtypes

F8 = ml_dtypes.float8_e4m3
BF16 = ml_dtypes.bfloat16
N = 262144
D = 1024
H = 512
B = 512
NCORES = 8
ROWS = N // NCORES
P = 128
MAXB = 128
KC2 = D // 256              # 4 double-row chunks

_CACHE = {}
TRACE = False
LAST_RESULT = None


def _build_program(n_tiles):
    import concourse.bass as bass
    import concourse.bacc as bacc
    import concourse.mybir as mybir
    import concourse.tile as tile

    dt = mybir.dt
    AF = mybir.ActivationFunctionType
    ALU = mybir.AluOpType
    DR = mybir.MatmulPerfMode.DoubleRow

    nc = bacc.Bacc("TRN2", target_bir_lowering=False, debug=False,
                   num_devices=NCORES)

    rows = n_tiles * P
    xp = nc.dram_tensor("xp", (n_tiles // 2, P, 2 * D), dt.bfloat16,
                        kind="ExternalInput")
    # DR-packed transposed x: [t, p, 2c+i, m] = x[t*128+m, c*256+i*128+p]/4
    xpt = nc.dram_tensor("xpt", (n_tiles, P, D), dt.float8e4,
                         kind="ExternalInput")
    oneh = nc.dram_tensor("oneh", (n_tiles // 2, P, 2 * MAXB), dt.bfloat16,
                          kind="ExternalInput")
    w12s = nc.dram_tensor("w12s", (P, 2 * KC2 * 2 * H), dt.float8e4,
                          kind="ExternalInput")
    w3r = nc.dram_tensor("w3r", (P, H), dt.float16, kind="ExternalInput")
    S = nc.dram_tensor("S", (MAXB, D), dt.float32, kind="ExternalOutput")
    E = nc.dram_tensor("E", (P, n_tiles), dt.float32, kind="ExternalOutput")

    with tile.TileContext(nc) as tc:
        with (
            tc.tile_pool(name="const", bufs=1) as constp,
            tc.tile_pool(name="xt", bufs=12) as xtp,
            tc.tile_pool(name="xn", bufs=8) as xnp_,
            tc.tile_pool(name="oh", bufs=8) as ohp,
            tc.tile_pool(name="work", bufs=4) as workp,
            tc.tile_pool(name="uvps", bufs=3, space=bass.MemorySpace.PSUM) as psp,
            tc.tile_pool(name="accps", bufs=1, space=bass.MemorySpace.PSUM) as psaccp,
        ):
            w12 = constp.tile([P, 2 * KC2, 2 * H], dt.float8e4)
            nc.gpsimd.dma_start(w12[:], w12s.ap())
            w3 = constp.tile([P, H], dt.float16)
            nc.gpsimd.dma_start(w3[:], w3r.ap())
            ebuf = constp.tile([P, n_tiles], dt.float32)
            pool_acc = psaccp.tile([MAXB, D], dt.float32)

            xn = oh = None
            for t in range(n_tiles):
                xt = xtp.tile([P, 2 * KC2, P], dt.float8e4)
                nc.sync.dma_start(xt[:], xpt[t])
                if t % 2 == 0:
                    xn = xnp_.tile([P, 2, D], dt.bfloat16)
                    nc.gpsimd.dma_start(xn[:], xp[t // 2])
                    oh = ohp.tile([P, 2, MAXB], dt.bfloat16)
                    nc.gpsimd.dma_start(oh[:], oneh[t // 2])

                uv = psp.tile([P, 2 * H], dt.float32)
                for c in range(KC2):
                    lhs = xt[:, 2 * c:2 * c + 2, :]
                    nc.tensor.matmul(uv[:, 0:H], lhs,
                                     w12[:, 2 * c:2 * c + 2, 0:H],
                                     start=(c == 0), stop=(c == KC2 - 1),
                                     perf_mode=DR)
                    nc.tensor.matmul(uv[:, H:2 * H], lhs,
                                     w12[:, 2 * c:2 * c + 2, H:2 * H],
                                     start=(c == 0), stop=(c == KC2 - 1),
                                     perf_mode=DR)

                u16 = workp.tile([P, H], dt.float16)
                nc.scalar.activation(u16[:], uv[:, 0:H], AF.Tanh)
                ev = workp.tile([P, H], dt.float16)
                den = workp.tile([P, 1], dt.float32)
                nc.scalar.activation(ev[:], uv[:, H:2 * H], AF.Exp,
                                     accum_out=den[:])
                uw = workp.tile([P, H], dt.float16)
                nc.vector.tensor_tensor(uw[:], u16[:], w3[:], ALU.mult)
                prod = workp.tile([P, H], dt.float16)
                nc.vector.tensor_tensor(prod[:], uw[:], ev[:], ALU.mult)
                num = workp.tile([P, 1], dt.float32)
                nc.vector.reduce_sum(num[:], prod[:], mybir.AxisListType.X)
                rden = workp.tile([P, 1], dt.float32)
                nc.vector.reciprocal(rden[:], den[:])
                nc.scalar.activation(ebuf[:, t:t + 1], num[:], AF.Exp,
                                     scale=rden[:])
                lhsp = workp.tile([P, MAXB], dt.bfloat16)
                nc.vector.tensor_scalar_mul(lhsp[:], oh[:, t % 2, :],
                                            ebuf[:, t:t + 1])
                nc.tensor.matmul(pool_acc[:, 0:H], lhsp[:],
                                 xn[:, t % 2, 0:H],
                                 start=(t == 0), stop=(t == n_tiles - 1),
                                 skip_group_check=True)
                nc.tensor.matmul(pool_acc[:, H:D], lhsp[:],
                                 xn[:, t % 2, H:D],
                                 start=(t == 0), stop=(t == n_tiles - 1),
                                 skip_group_check=True)

            sout = constp.tile([MAXB, D], dt.float32)
            nc.scalar.copy(sout[:], pool_acc[:])
            nc.gpsimd.dma_start(S.ap(), sout[:])
            nc.gpsimd.dma_start(E.ap(), ebuf[:])

    nc.compile()
    return nc


def _get_program(n_tiles):
    if n_tiles not in _CACHE:
        _CACHE[n_tiles] = _build_program(n_tiles)
    return _CACHE[n_tiles]


def kernel(x, batch, W1, W2, W3):
    global LAST_RESULT
    from concourse import bass_utils

    x = np.asarray(x, dtype=np.float32)
    batch = np.asarray(batch)
    W1 = np.asarray(W1, dtype=np.float32)
    W2 = np.asarray(W2, dtype=np.float32)
    W3 = np.asarray(W3, dtype=np.float32)

    w12t = np.concatenate([W1.T, W2.T], axis=1)              # (D, 2H)
    w12dr = np.ascontiguousarray(
        (w12t * 4.0).reshape(KC2, 2, P, 2 * H).transpose(2, 0, 1, 3)
        .reshape(P, -1)).astype(F8)
    w3r = np.ascontiguousarray(
        np.broadcast_to(W3.reshape(1, H), (P, H))).astype(np.float16)

    x8 = (x * 0.25).astype(F8)
    x16 = x.astype(BF16)

    n_tiles = ROWS // P
    in_maps = []
    bases = []
    locals_ = []
    for c in range(NCORES):
        ids = batch[c * ROWS:(c + 1) * ROWS].astype(np.int64)
        base = int(ids[0])
        local = (ids - base).astype(np.int64)
        nb = int(local.max()) + 1
        assert nb <= MAXB, f"core {c}: {nb} local bags > {MAXB}"
        oneh = np.zeros((ROWS, MAXB), dtype=BF16)
        oneh[np.arange(ROWS), local] = BF16(1.0)
        xs8 = x8[c * ROWS:(c + 1) * ROWS]
        xt8 = np.ascontiguousarray(
            xs8.reshape(n_tiles, P, KC2, 2, P).transpose(0, 4, 2, 3, 1)
            .reshape(n_tiles, P, D))
        xs16 = x16[c * ROWS:(c + 1) * ROWS]
        xp2 = np.ascontiguousarray(
            xs16.reshape(n_tiles // 2, 2, P, D).transpose(0, 2, 1, 3)
            .reshape(n_tiles // 2, P, 2 * D))
        oh2 = np.ascontiguousarray(
            oneh.reshape(n_tiles // 2, 2, P, MAXB).transpose(0, 2, 1, 3)
            .reshape(n_tiles // 2, P, 2 * MAXB))
        in_maps.append({
            "xp": xp2,
            "xpt": xt8,
            "oneh": oh2,
            "w12s": w12dr,
            "w3r": w3r,
        })
        bases.append(base)
        locals_.append(local)

    nc = _get_program(n_tiles)
    res = bass_utils.run_bass_kernel_spmd(
        nc, in_maps, core_ids=list(range(NCORES)), trace=TRACE)
    LAST_RESULT = res

    Z = np.zeros((B, D), dtype=np.float64)
    DEN = np.zeros((B,), dtype=np.float64)
    for c in range(NCORES):
        Sc = np.asarray(res.results[c]["S"], dtype=np.float64)
        Ec = np.asarray(res.results[c]["E"], dtype=np.float64)
        e_flat = Ec.T.reshape(-1)
        local = locals_[c]
        nb = int(local.max()) + 1
        den = np.bincount(local, weights=e_flat, minlength=nb)[:nb]
        Z[bases[c]:bases[c] + nb] += Sc[:nb]
        DEN[bases[c]:bases[c] + nb] += den
    out = np.zeros((B, D), dtype=np.float32)
    nzero = DEN > 0
    out[nzero] = (Z[nzero] / DEN[nzero, None]).astype(np.float32)
    return out
